# revision 19
# baseline (speedup 1.0000x reference)
"""DiagSSMBlock Trainium2 kernel.

Math (matches the reference exactly):
    s = b_mat.T @ x_seq.T                  # (H, T)
    y[h, t] = a[h] * y[h, t-1] + s[h, t]   # first-order IIR scan along t
    out = y.T                              # (T, H)

Sharding: a 2 (H) x 4 (T) grid over 8 cores. Each core computes a
(1024 channels x 1024 timesteps) output block: a (2048x1024)^T @
(2048x1024) matmul accumulated over K=2048 in PSUM, then the
per-channel IIR scan via the Vector engine's tensor_tensor_scan.

Time-sharding needs no cross-core communication: |a| <= sqrt(2/2048)
~ 0.031, so the scan state decays below fp32 noise within a few steps.
Each core's scan is seeded with a carry computed on the host from a
16-column warm-up strip (a^17 ~ 1e-25 of history is dropped -- exactly
zero in fp32). The strip matmul is 0.1% of the device FLOPs.

x is fed pre-transposed (K-major) from the host so both matmul operands
have the contraction dim in partitions; no on-chip transpose needed.
The matmul runs in float32r (full-rate relaxed fp32, ~1.5e-4 rel err);
set MM_DTYPE to float32 for the exact (4x slower) variant.
"""

import sys

import numpy as np

_REPO = "/opt/trn_rl_repo"
if _REPO not in sys.path:
    sys.path.insert(0, _REPO)

import concourse.bass as bass
import concourse.mybir as mybir
from concourse import bacc
from concourse.bass_utils import run_bass_kernel_spmd
from concourse.tile import TileContext

T = 4096
H = 2048
NCORES = 8
HG = 2           # h groups
TG = 4           # t groups
HSH = H // HG    # 1024 channels per core
TSH = T // TG    # 1024 timesteps per core
WARM = 16        # host-side scan warm-up columns per t boundary
P = 128
KT = H // P      # 16 k-tiles
MT = HSH // P    # 8 m-tiles
NCH = 512
CHUNKS = ((0, NCH), (NCH, NCH))  # matmul/scan t-chunks per core

MM_DTYPE = mybir.dt.float32r

_nc_cache = {}


def build_nc(mm_dtype=MM_DTYPE):
    f32 = mybir.dt.float32
    nc = bacc.Bacc(None, target_bir_lowering=False)

    xt = nc.declare_dram_parameter("xt", [H, TSH], mm_dtype, isOutput=False)
    bm = nc.declare_dram_parameter("bm", [H, HSH], mm_dtype, isOutput=False)
    av = nc.declare_dram_parameter("av", [HSH], f32, isOutput=False)
    cv = nc.declare_dram_parameter("cv", [HSH], f32, isOutput=False)
    y = nc.declare_dram_parameter("y", [HSH, TSH], f32, isOutput=True)

    xt_r = xt.rearrange("(ko p) t -> p ko t", p=P)  # [128, 16, 1024]
    bm_r = bm.rearrange("(ko p) m -> p ko m", p=P)  # [128, 16, 1024]
    av_r = av.rearrange("(mo p) -> p mo", p=P)      # [128, 8]
    cv_r = cv.rearrange("(mo p) -> p mo", p=P)      # [128, 8]
    y_r = y.rearrange("(mo p) t -> p mo t", p=P)    # [128, 8, 1024]

    NPAIR = MT // 2
    with TileContext(nc) as tc:
        with (
            tc.tile_pool(name="const", bufs=1) as cpool,
            tc.tile_pool(name="xp", bufs=KT) as xpool,
            tc.tile_pool(name="bp", bufs=KT * NPAIR) as bpool,
            tc.tile_pool(name="yp", bufs=MT) as ypool,
            tc.tile_pool(name="ps0", bufs=4, space="PSUM") as p0pool,
            tc.tile_pool(name="ps1", bufs=4, space="PSUM") as p1pool,
        ):
            # x k-tiles interleaved with the b slices the first two
            # m-pairs need, so the PE starts as soon as tiles land.
            x_tiles = []
            b_tiles = {}
            a_sb = cpool.tile([P, MT], f32)
            c_sb = cpool.tile([P, MT], f32)
            for k in range(KT):
                xk = xpool.tile([P, TSH], mm_dtype, tag="x")
                nc.sync.dma_start(out=xk[:], in_=xt_r[:, k, :])
                x_tiles.append(xk)
                for mp in range(2):
                    bk = bpool.tile([P, 2 * P], mm_dtype, tag="b")
                    nc.sync.dma_start(
                        out=bk[:], in_=bm_r[:, k, mp * 2 * P : (mp + 1) * 2 * P]
                    )
                    b_tiles[(k, mp)] = bk
                if k == 0:
                    # a/c are only needed by the first scan, well after
                    # the stream-critical first x/b tiles.
                    nc.sync.dma_start(out=a_sb[:], in_=av_r[:])
                    nc.sync.dma_start(out=c_sb[:], in_=cv_r[:])
            for mp in range(2, NPAIR):
                for k in range(KT):
                    bk = bpool.tile([P, 2 * P], mm_dtype, tag="b")
                    nc.sync.dma_start(
                        out=bk[:], in_=bm_r[:, k, mp * 2 * P : (mp + 1) * 2 * P]
                    )
                    b_tiles[(k, mp)] = bk

            def emit_scans(m, pst, ym):
                for ci, (c0, cw) in enumerate(CHUNKS):
                    nc.vector.tensor_tensor_scan(
                        out=ym[:, c0 : c0 + cw],
                        data0=a_sb[:, m : m + 1].broadcast_to((P, cw)),
                        data1=pst[ci][:],
                        initial=(
                            c_sb[:, m : m + 1] if ci == 0 else ym[:, c0 - 1 : c0]
                        ),
                        op0=mybir.AluOpType.mult,
                        op1=mybir.AluOpType.add,
                    )
                nc.scalar.dma_start(out=y_r[:, m, :], in_=ym[:])

            def psum_tiles():
                t0 = p0pool.tile([P, NCH], f32, tag="ps0")
                t1 = p1pool.tile([P, NCH], f32, tag="ps1")
                return (t0, t1)

            # Phase 1: m-tiles 0-3 interleaved k-major across all 8 PSUM
            # banks -- enough PE work per arriving x k-tile to keep the
            # PE busy while the input stream is still landing.
            pss1 = [psum_tiles() for _ in range(4)]
            for k in range(KT):
                for mi in range(4):
                    lhsT = b_tiles[(k, mi // 2)][:, (mi % 2) * P : (mi % 2 + 1) * P]
                    for ci, (c0, cw) in enumerate(CHUNKS):
                        nc.tensor.matmul(
                            pss1[mi][ci][:],
                            lhsT,
                            x_tiles[k][:, c0 : c0 + cw],
                            start=(k == 0),
                            stop=(k == KT - 1),
                        )
            for mi in range(4):
                ym = ypool.tile([P, TSH], f32, tag="y")
                emit_scans(mi, pss1[mi], ym)

            # Phase 2: m-tiles 4-7. Pair 2 k-major; the final pair is
            # m-major/chunk-major so all but the very last scan overlap
            # remaining matmuls, shrinking the kernel tail.
            pss2 = [psum_tiles() for _ in range(2)]
            for k in range(KT):
                for m2 in range(2):
                    lhsT = b_tiles[(k, 2)][:, m2 * P : (m2 + 1) * P]
                    for ci, (c0, cw) in enumerate(CHUNKS):
                        nc.tensor.matmul(
                            pss2[m2][ci][:],
                            lhsT,
                            x_tiles[k][:, c0 : c0 + cw],
                            start=(k == 0),
                            stop=(k == KT - 1),
                        )
            for m2 in range(2):
                ym = ypool.tile([P, TSH], f32, tag="y")
                emit_scans(4 + m2, pss2[m2], ym)

            for m2 in range(2):
                pst = psum_tiles()
                for ci, (c0, cw) in enumerate(CHUNKS):
                    for k in range(KT):
                        lhsT = b_tiles[(k, 3)][:, m2 * P : (m2 + 1) * P]
                        nc.tensor.matmul(
                            pst[ci][:],
                            lhsT,
                            x_tiles[k][:, c0 : c0 + cw],
                            start=(k == 0),
                            stop=(k == KT - 1),
                        )
                ym = ypool.tile([P, TSH], f32, tag="y")
                emit_scans(6 + m2, pst, ym)
    nc.finalize()
    return nc


def make_in_maps(x_seq, a_diag, b_mat):
    x_seq = np.ascontiguousarray(np.asarray(x_seq, dtype=np.float32))
    a_diag = np.ascontiguousarray(np.asarray(a_diag, dtype=np.float32))
    b_mat = np.ascontiguousarray(np.asarray(b_mat, dtype=np.float32))
    assert x_seq.shape == (T, H) and a_diag.shape == (H,) and b_mat.shape == (H, H)

    np_mm = mybir.dt.np(MM_DTYPE)
    xT = np.ascontiguousarray(x_seq.T)  # (H, T), K-major for the PE

    # Scan warm-up carries at each t-block boundary: scan a 16-column
    # strip of s = b^T x from zero state. History older than the strip
    # contributes < |a|^17 ~ 1e-25 relative -- exactly zero in fp32.
    carries = np.zeros((TG, H), dtype=np.float32)
    for tg in range(1, TG):
        strip = b_mat.T @ xT[:, tg * TSH - WARM : tg * TSH]  # (H, WARM)
        state = np.zeros(H, dtype=np.float32)
        for j in range(WARM):
            state = a_diag * state + strip[:, j]
        carries[tg] = state

    in_maps = []
    for c in range(NCORES):
        hg, tg = divmod(c, TG)
        hsl = slice(hg * HSH, (hg + 1) * HSH)
        in_maps.append(
            {
                "xt": np.ascontiguousarray(xT[:, tg * TSH : (tg + 1) * TSH]).astype(
                    np_mm
                ),
                "bm": np.ascontiguousarray(b_mat[:, hsl]).astype(np_mm),
                "av": np.ascontiguousarray(a_diag[hsl]),
                "cv": np.ascontiguousarray(carries[tg, hsl]),
            }
        )
    return in_maps


def run(in_maps, **kwargs):
    key = MM_DTYPE
    if key not in _nc_cache:
        _nc_cache[key] = build_nc(key)
    return run_bass_kernel_spmd(_nc_cache[key], in_maps, list(range(NCORES)), **kwargs)


def kernel(x_seq, a_diag, b_mat):
    res = run(make_in_maps(x_seq, a_diag, b_mat))
    yT = np.empty((H, T), dtype=np.float32)
    for c in range(NCORES):
        hg, tg = divmod(c, TG)
        yT[hg * HSH : (hg + 1) * HSH, tg * TSH : (tg + 1) * TSH] = res.results[c]["y"]
    return np.ascontiguousarray(yT.T)


# revision 20
# speedup vs baseline: 1.0593x; 1.0593x over previous
"""DiagSSMBlock Trainium2 kernel.

Math (matches the reference exactly):
    s = b_mat.T @ x_seq.T                  # (H, T)
    y[h, t] = a[h] * y[h, t-1] + s[h, t]   # first-order IIR scan along t
    out = y.T                              # (T, H)

Sharding: a 2 (H) x 4 (T) grid over 8 cores. Each core computes a
(1024 channels x 1024 timesteps) output block: a (2048x1024)^T @
(2048x1024) matmul accumulated over K=2048 in PSUM, then the
per-channel IIR scan via the Vector engine's tensor_tensor_scan.

Time-sharding needs no cross-core communication: |a| <= sqrt(2/2048)
~ 0.031, so the scan state decays below fp32 noise within a few steps.
Each core's scan is seeded with a carry computed on the host from a
16-column warm-up strip (a^17 ~ 1e-25 of history is dropped -- exactly
zero in fp32). The strip matmul is 0.1% of the device FLOPs.

x is fed pre-transposed (K-major) from the host so both matmul operands
have the contraction dim in partitions; no on-chip transpose needed.
The matmul runs in float32r (full-rate relaxed fp32, ~1.5e-4 rel err);
set MM_DTYPE to float32 for the exact (4x slower) variant.
"""

import sys

import numpy as np

_REPO = "/opt/trn_rl_repo"
if _REPO not in sys.path:
    sys.path.insert(0, _REPO)

import concourse.bass as bass
import concourse.mybir as mybir
from concourse import bacc
from concourse.bass_utils import run_bass_kernel_spmd
from concourse.tile import TileContext

T = 4096
H = 2048
NCORES = 8
HG = 2           # h groups
TG = 4           # t groups
HSH = H // HG    # 1024 channels per core
TSH = T // TG    # 1024 timesteps per core
WARM = 16        # host-side scan warm-up columns per t boundary
P = 128
KT = H // P      # 16 k-tiles
MT = HSH // P    # 8 m-tiles
NCH = 512
CHUNKS = ((0, NCH), (NCH, NCH))  # matmul/scan t-chunks per core

MM_DTYPE = mybir.dt.float32r

_nc_cache = {}


def build_nc(mm_dtype=MM_DTYPE):
    f32 = mybir.dt.float32
    nc = bacc.Bacc(None, target_bir_lowering=False)

    xt = nc.declare_dram_parameter("xt", [H, TSH], mm_dtype, isOutput=False)
    bm = nc.declare_dram_parameter("bm", [H, HSH], mm_dtype, isOutput=False)
    av = nc.declare_dram_parameter("av", [HSH], f32, isOutput=False)
    cv = nc.declare_dram_parameter("cv", [HSH], f32, isOutput=False)
    y = nc.declare_dram_parameter("y", [HSH, TSH], f32, isOutput=True)

    xt_r = xt.rearrange("(ko p) t -> p ko t", p=P)  # [128, 16, 1024]
    bm_r = bm.rearrange("(ko p) m -> p ko m", p=P)  # [128, 16, 1024]
    av_r = av.rearrange("(mo p) -> p mo", p=P)      # [128, 8]
    cv_r = cv.rearrange("(mo p) -> p mo", p=P)      # [128, 8]
    y_r = y.rearrange("(mo p) t -> p mo t", p=P)    # [128, 8, 1024]

    NPAIR = MT // 2
    with TileContext(nc) as tc:
        with (
            tc.tile_pool(name="const", bufs=1) as cpool,
            tc.tile_pool(name="xp", bufs=KT) as xpool,
            tc.tile_pool(name="bp", bufs=KT * NPAIR) as bpool,
            tc.tile_pool(name="yp", bufs=MT) as ypool,
            tc.tile_pool(name="ps0", bufs=4, space="PSUM") as p0pool,
            tc.tile_pool(name="ps1", bufs=4, space="PSUM") as p1pool,
        ):
            a_sb = cpool.tile([P, MT], f32)
            c_sb = cpool.tile([P, MT], f32)

            # x k-tiles interleaved with the b slices the first m-pair
            # needs, so the PE starts as soon as tiles land.
            x_tiles = []
            b_tiles = {}
            for k in range(KT):
                xk = xpool.tile([P, TSH], mm_dtype, tag="x")
                nc.sync.dma_start(out=xk[:], in_=xt_r[:, k, :])
                x_tiles.append(xk)
                bk = bpool.tile([P, 2 * P], mm_dtype, tag="b")
                nc.sync.dma_start(out=bk[:], in_=bm_r[:, k, 0 : 2 * P])
                b_tiles[(k, 0)] = bk
                if k == 0:
                    # a/c are only needed by the first scan, well after
                    # the stream-critical first x/b tiles.
                    nc.sync.dma_start(out=a_sb[:], in_=av_r[:])
                    nc.sync.dma_start(out=c_sb[:], in_=cv_r[:])
            for mp in range(1, NPAIR):
                for k in range(KT):
                    bk = bpool.tile([P, 2 * P], mm_dtype, tag="b")
                    nc.sync.dma_start(
                        out=bk[:], in_=bm_r[:, k, mp * 2 * P : (mp + 1) * 2 * P]
                    )
                    b_tiles[(k, mp)] = bk

            for mp in range(NPAIR):
                pss = []
                for m2 in range(2):
                    p0 = p0pool.tile([P, NCH], f32, tag="ps0")
                    p1 = p1pool.tile([P, NCH], f32, tag="ps1")
                    pss.append((p0, p1))
                if mp < NPAIR - 1:
                    # k-major: chases the initial x/b DMA stream
                    for k in range(KT):
                        for m2 in range(2):
                            lhsT = b_tiles[(k, mp)][:, m2 * P : (m2 + 1) * P]
                            for ci, (c0, cw) in enumerate(CHUNKS):
                                nc.tensor.matmul(
                                    pss[m2][ci][:],
                                    lhsT,
                                    x_tiles[k][:, c0 : c0 + cw],
                                    start=(k == 0),
                                    stop=(k == KT - 1),
                                )
                else:
                    # Last pair: m-major and chunk-major, so every scan
                    # except the very last overlaps remaining matmuls,
                    # shrinking the kernel tail.
                    for m2 in range(2):
                        for ci, (c0, cw) in enumerate(CHUNKS):
                            for k in range(KT):
                                lhsT = b_tiles[(k, mp)][:, m2 * P : (m2 + 1) * P]
                                nc.tensor.matmul(
                                    pss[m2][ci][:],
                                    lhsT,
                                    x_tiles[k][:, c0 : c0 + cw],
                                    start=(k == 0),
                                    stop=(k == KT - 1),
                                )
                        m = 2 * mp + m2
                        ym = ypool.tile([P, TSH], f32, tag="ylast")
                        for ci, (c0, cw) in enumerate(CHUNKS):
                            nc.vector.tensor_tensor_scan(
                                out=ym[:, c0 : c0 + cw],
                                data0=a_sb[:, m : m + 1].broadcast_to((P, cw)),
                                data1=pss[m2][ci][:],
                                initial=(
                                    c_sb[:, m : m + 1]
                                    if ci == 0
                                    else ym[:, c0 - 1 : c0]
                                ),
                                op0=mybir.AluOpType.mult,
                                op1=mybir.AluOpType.add,
                            )
                        nc.scalar.dma_start(out=y_r[:, m, :], in_=ym[:])
                if mp < NPAIR - 1:
                    for m2 in range(2):
                        m = 2 * mp + m2
                        ym = ypool.tile([P, TSH], f32, tag="y")
                        for ci, (c0, cw) in enumerate(CHUNKS):
                            nc.vector.tensor_tensor_scan(
                                out=ym[:, c0 : c0 + cw],
                                data0=a_sb[:, m : m + 1].broadcast_to((P, cw)),
                                data1=pss[m2][ci][:],
                                initial=(
                                    c_sb[:, m : m + 1]
                                    if ci == 0
                                    else ym[:, c0 - 1 : c0]
                                ),
                                op0=mybir.AluOpType.mult,
                                op1=mybir.AluOpType.add,
                            )
                        nc.scalar.dma_start(out=y_r[:, m, :], in_=ym[:])
    nc.finalize()
    return nc


def make_in_maps(x_seq, a_diag, b_mat):
    x_seq = np.ascontiguousarray(np.asarray(x_seq, dtype=np.float32))
    a_diag = np.ascontiguousarray(np.asarray(a_diag, dtype=np.float32))
    b_mat = np.ascontiguousarray(np.asarray(b_mat, dtype=np.float32))
    assert x_seq.shape == (T, H) and a_diag.shape == (H,) and b_mat.shape == (H, H)

    xT = np.ascontiguousarray(x_seq.T)  # (H, T), K-major for the PE

    # Scan warm-up carries at each t-block boundary: scan a 16-column
    # strip of s = b^T x from zero state. History older than the strip
    # contributes < |a|^17 ~ 1e-25 relative -- exactly zero in fp32.
    carries = np.zeros((TG, H), dtype=np.float32)
    for tg in range(1, TG):
        strip = b_mat.T @ xT[:, tg * TSH - WARM : tg * TSH]  # (H, WARM)
        state = np.zeros(H, dtype=np.float32)
        for j in range(WARM):
            state = a_diag * state + strip[:, j]
        carries[tg] = state

    in_maps = []
    for c in range(NCORES):
        hg, tg = divmod(c, TG)
        hsl = slice(hg * HSH, (hg + 1) * HSH)
        in_maps.append(
            {
                "xt": np.ascontiguousarray(xT[:, tg * TSH : (tg + 1) * TSH]),
                "bm": np.ascontiguousarray(b_mat[:, hsl]),
                "av": np.ascontiguousarray(a_diag[hsl]),
                "cv": np.ascontiguousarray(carries[tg, hsl]),
            }
        )
    return in_maps


def run(in_maps, **kwargs):
    key = MM_DTYPE
    if key not in _nc_cache:
        _nc_cache[key] = build_nc(key)
    return run_bass_kernel_spmd(_nc_cache[key], in_maps, list(range(NCORES)), **kwargs)


def kernel(x_seq, a_diag, b_mat):
    res = run(make_in_maps(x_seq, a_diag, b_mat))
    yT = np.empty((H, T), dtype=np.float32)
    for c in range(NCORES):
        hg, tg = divmod(c, TG)
        yT[hg * HSH : (hg + 1) * HSH, tg * TSH : (tg + 1) * TSH] = res.results[c]["y"]
    return np.ascontiguousarray(yT.T)


# revision 21
# speedup vs baseline: 1.0640x; 1.0045x over previous
"""DiagSSMBlock Trainium2 kernel.

Math (matches the reference exactly):
    s = b_mat.T @ x_seq.T                  # (H, T)
    y[h, t] = a[h] * y[h, t-1] + s[h, t]   # first-order IIR scan along t
    out = y.T                              # (T, H)

Sharding: a 2 (H) x 4 (T) grid over 8 cores. Each core computes a
(1024 channels x 1024 timesteps) output block: a (2048x1024)^T @
(2048x1024) matmul accumulated over K=2048 in PSUM, then the
per-channel IIR scan via the Vector engine's tensor_tensor_scan.

Time-sharding needs no cross-core communication: |a| <= sqrt(2/2048)
~ 0.031, so the scan state decays below fp32 noise within a few steps.
Each core's scan is seeded with a carry computed on the host from a
16-column warm-up strip (a^17 ~ 1e-25 of history is dropped -- exactly
zero in fp32). The strip matmul is 0.1% of the device FLOPs.

x is fed pre-transposed (K-major) from the host so both matmul operands
have the contraction dim in partitions; no on-chip transpose needed.
The matmul runs in float32r (full-rate relaxed fp32, ~1.5e-4 rel err);
set MM_DTYPE to float32 for the exact (4x slower) variant.
"""

import sys

import numpy as np

_REPO = "/opt/trn_rl_repo"
if _REPO not in sys.path:
    sys.path.insert(0, _REPO)

import concourse.bass as bass
import concourse.mybir as mybir
from concourse import bacc
from concourse.bass_utils import run_bass_kernel_spmd
from concourse.tile import TileContext

T = 4096
H = 2048
NCORES = 8
HG = 2           # h groups
TG = 4           # t groups
HSH = H // HG    # 1024 channels per core
TSH = T // TG    # 1024 timesteps per core
WARM = 16        # host-side scan warm-up columns per t boundary
P = 128
KT = H // P      # 16 k-tiles
MT = HSH // P    # 8 m-tiles
NCH = 512
CHUNKS = ((0, NCH), (NCH, NCH))  # matmul/scan t-chunks per core

MM_DTYPE = mybir.dt.float32r

_nc_cache = {}


def build_nc(mm_dtype=MM_DTYPE):
    f32 = mybir.dt.float32
    nc = bacc.Bacc(None, target_bir_lowering=False)

    xt = nc.declare_dram_parameter("xt", [H, TSH], mm_dtype, isOutput=False)
    bm = nc.declare_dram_parameter("bm", [H, HSH], mm_dtype, isOutput=False)
    av = nc.declare_dram_parameter("av", [HSH], f32, isOutput=False)
    cv = nc.declare_dram_parameter("cv", [HSH], f32, isOutput=False)
    y = nc.declare_dram_parameter("y", [HSH, TSH], f32, isOutput=True)

    xt_r = xt.rearrange("(ko p) t -> p ko t", p=P)  # [128, 16, 1024]
    bm_r = bm.rearrange("(ko p) m -> p ko m", p=P)  # [128, 16, 1024]
    av_r = av.rearrange("(mo p) -> p mo", p=P)      # [128, 8]
    cv_r = cv.rearrange("(mo p) -> p mo", p=P)      # [128, 8]
    y_r = y.rearrange("(mo p) t -> p mo t", p=P)    # [128, 8, 1024]

    NPAIR = MT // 2
    with TileContext(nc) as tc:
        with (
            tc.tile_pool(name="const", bufs=1) as cpool,
            tc.tile_pool(name="xp", bufs=KT) as xpool,
            tc.tile_pool(name="bp", bufs=KT * NPAIR) as bpool,
            tc.tile_pool(name="yp", bufs=MT) as ypool,
            tc.tile_pool(name="ps0", bufs=4, space="PSUM") as p0pool,
            tc.tile_pool(name="ps1", bufs=4, space="PSUM") as p1pool,
        ):
            a_sb = cpool.tile([P, MT], f32)
            c_sb = cpool.tile([P, MT], f32)

            # x k-tiles interleaved with the b slices the first m-pair
            # needs, so the PE starts as soon as tiles land.
            x_tiles = []
            b_tiles = {}
            for k in range(KT):
                xk = xpool.tile([P, TSH], mm_dtype, tag="x")
                # the first tiles go out on the gpsimd (SWDGE) queue,
                # which is free before the sync queue's preamble table
                # loads finish
                xq = nc.gpsimd if k < 2 else nc.sync
                xq.dma_start(out=xk[:], in_=xt_r[:, k, :])
                x_tiles.append(xk)
                bk = bpool.tile([P, 2 * P], mm_dtype, tag="b")
                xq.dma_start(out=bk[:], in_=bm_r[:, k, 0 : 2 * P])
                b_tiles[(k, 0)] = bk
                if k == 0:
                    # a/c are only needed by the first scan, well after
                    # the stream-critical first x/b tiles.
                    nc.sync.dma_start(out=a_sb[:], in_=av_r[:])
                    nc.sync.dma_start(out=c_sb[:], in_=cv_r[:])
            for mp in range(1, NPAIR):
                for k in range(KT):
                    bk = bpool.tile([P, 2 * P], mm_dtype, tag="b")
                    nc.sync.dma_start(
                        out=bk[:], in_=bm_r[:, k, mp * 2 * P : (mp + 1) * 2 * P]
                    )
                    b_tiles[(k, mp)] = bk

            for mp in range(NPAIR):
                pss = []
                for m2 in range(2):
                    p0 = p0pool.tile([P, NCH], f32, tag="ps0")
                    p1 = p1pool.tile([P, NCH], f32, tag="ps1")
                    pss.append((p0, p1))
                if mp < NPAIR - 1:
                    # k-major: chases the initial x/b DMA stream
                    for k in range(KT):
                        for m2 in range(2):
                            lhsT = b_tiles[(k, mp)][:, m2 * P : (m2 + 1) * P]
                            for ci, (c0, cw) in enumerate(CHUNKS):
                                nc.tensor.matmul(
                                    pss[m2][ci][:],
                                    lhsT,
                                    x_tiles[k][:, c0 : c0 + cw],
                                    start=(k == 0),
                                    stop=(k == KT - 1),
                                )
                else:
                    # Last pair: m-major and chunk-major, so every scan
                    # except the very last overlaps remaining matmuls,
                    # shrinking the kernel tail.
                    for m2 in range(2):
                        for ci, (c0, cw) in enumerate(CHUNKS):
                            for k in range(KT):
                                lhsT = b_tiles[(k, mp)][:, m2 * P : (m2 + 1) * P]
                                nc.tensor.matmul(
                                    pss[m2][ci][:],
                                    lhsT,
                                    x_tiles[k][:, c0 : c0 + cw],
                                    start=(k == 0),
                                    stop=(k == KT - 1),
                                )
                        m = 2 * mp + m2
                        ym = ypool.tile([P, TSH], f32, tag="ylast")
                        for ci, (c0, cw) in enumerate(CHUNKS):
                            nc.vector.tensor_tensor_scan(
                                out=ym[:, c0 : c0 + cw],
                                data0=a_sb[:, m : m + 1].broadcast_to((P, cw)),
                                data1=pss[m2][ci][:],
                                initial=(
                                    c_sb[:, m : m + 1]
                                    if ci == 0
                                    else ym[:, c0 - 1 : c0]
                                ),
                                op0=mybir.AluOpType.mult,
                                op1=mybir.AluOpType.add,
                            )
                        nc.scalar.dma_start(out=y_r[:, m, :], in_=ym[:])
                if mp < NPAIR - 1:
                    for m2 in range(2):
                        m = 2 * mp + m2
                        ym = ypool.tile([P, TSH], f32, tag="y")
                        for ci, (c0, cw) in enumerate(CHUNKS):
                            nc.vector.tensor_tensor_scan(
                                out=ym[:, c0 : c0 + cw],
                                data0=a_sb[:, m : m + 1].broadcast_to((P, cw)),
                                data1=pss[m2][ci][:],
                                initial=(
                                    c_sb[:, m : m + 1]
                                    if ci == 0
                                    else ym[:, c0 - 1 : c0]
                                ),
                                op0=mybir.AluOpType.mult,
                                op1=mybir.AluOpType.add,
                            )
                        nc.scalar.dma_start(out=y_r[:, m, :], in_=ym[:])
    nc.finalize()
    return nc


def make_in_maps(x_seq, a_diag, b_mat):
    x_seq = np.ascontiguousarray(np.asarray(x_seq, dtype=np.float32))
    a_diag = np.ascontiguousarray(np.asarray(a_diag, dtype=np.float32))
    b_mat = np.ascontiguousarray(np.asarray(b_mat, dtype=np.float32))
    assert x_seq.shape == (T, H) and a_diag.shape == (H,) and b_mat.shape == (H, H)

    xT = np.ascontiguousarray(x_seq.T)  # (H, T), K-major for the PE

    # Scan warm-up carries at each t-block boundary: scan a 16-column
    # strip of s = b^T x from zero state. History older than the strip
    # contributes < |a|^17 ~ 1e-25 relative -- exactly zero in fp32.
    carries = np.zeros((TG, H), dtype=np.float32)
    for tg in range(1, TG):
        strip = b_mat.T @ xT[:, tg * TSH - WARM : tg * TSH]  # (H, WARM)
        state = np.zeros(H, dtype=np.float32)
        for j in range(WARM):
            state = a_diag * state + strip[:, j]
        carries[tg] = state

    in_maps = []
    for c in range(NCORES):
        hg, tg = divmod(c, TG)
        hsl = slice(hg * HSH, (hg + 1) * HSH)
        in_maps.append(
            {
                "xt": np.ascontiguousarray(xT[:, tg * TSH : (tg + 1) * TSH]),
                "bm": np.ascontiguousarray(b_mat[:, hsl]),
                "av": np.ascontiguousarray(a_diag[hsl]),
                "cv": np.ascontiguousarray(carries[tg, hsl]),
            }
        )
    return in_maps


def run(in_maps, **kwargs):
    key = MM_DTYPE
    if key not in _nc_cache:
        _nc_cache[key] = build_nc(key)
    return run_bass_kernel_spmd(_nc_cache[key], in_maps, list(range(NCORES)), **kwargs)


def kernel(x_seq, a_diag, b_mat):
    res = run(make_in_maps(x_seq, a_diag, b_mat))
    yT = np.empty((H, T), dtype=np.float32)
    for c in range(NCORES):
        hg, tg = divmod(c, TG)
        yT[hg * HSH : (hg + 1) * HSH, tg * TSH : (tg + 1) * TSH] = res.results[c]["y"]
    return np.ascontiguousarray(yT.T)
